# revision 25
# baseline (speedup 1.0000x reference)
"""Trainium2 Bass kernel for nn_CrossLayer (DCN-style cross stack).

Reference semantics (B=16384, D=1024, L=8):
    out_0 = x
    s_i = einsum('bd,d->b', out_i, W[i])
    out_{i+1} = x * s_i[:, None] + b[i] + x

Algebraic collapse: out_{i+1} = x * rho_{i+1} + b[i] with
    rho_1 = u_0 + 1,   rho_{l+1} = rho_l * u_l + c_l
    u_l[r] = <x[r, :], W[l]>          (U = x @ W.T, [B, L])
    c_l = <b[l-1], W[l]> + 1          (weights-only scalars)
    out = x * rho_8[:, None] + b[L-1]

Device work: U = x @ W.T via PE transposes + W-stationary matmuls, all in
float32r (1-pass PE datapath: transpose 1.5 cyc/row, matmul 1 cyc/row at
>=256 moving cols, vs 2/4 for plain fp32), an 8-step per-row scan on DVE
(initial=1, c_0=1 folds the +1 into the scan), one fused scale+bias pass.
x read once, out written once -> memory-roofline bound (~47 us/core).

Sharding: data-parallel over batch; 8 cores x 2048 rows. Tiny (L, D)
weights replicated.
"""

import numpy as np

import concourse.bacc as bacc
import concourse.tile as tile
from concourse import mybir
from concourse.bass_utils import run_bass_kernel_spmd

N_CORES = 8
B, D, L = 16384, 1024, 8
RPC = B // N_CORES          # rows per core (2048)
NT = RPC // 128             # 128-row tiles per core (16)
NCH = D // 128              # 128-wide d chunks (8)
GROUPS = [4, 4, 4, 2, 2]    # tiles per group (sum = NT); small tail groups
N_WARM = 8                  # bf16 warmup matmuls to hold PE p-state up

LAST_RESULTS = None


def _build(cvals):
    """Trace + compile the per-core program. cvals = [c_1..c_{L-1}]."""
    nc = bacc.Bacc("TRN2", target_bir_lowering=False, debug=False)
    f32 = mybir.dt.float32
    f32r = mybir.dt.float32r
    bf16 = mybir.dt.bfloat16

    x_d = nc.dram_tensor("x", [RPC, D], f32, kind="ExternalInput")
    wt_d = nc.dram_tensor("wt", [128, NCH * L], f32, kind="ExternalInput")
    b7_d = nc.dram_tensor("b7r", [128, D], f32, kind="ExternalInput")
    id_d = nc.dram_tensor("ident", [128, 128], f32, kind="ExternalInput")
    y_d = nc.dram_tensor("y", [RPC, D], f32, kind="ExternalOutput")

    # tile views: [t][p, d]
    x_tile = x_d.ap().rearrange("(t p) d -> t p d", p=128)
    x_pair = x_d.ap().rearrange("(h t p) d -> h p t d", t=2, p=128)  # 1MB views
    y_tile = y_d.ap().rearrange("(t p) d -> t p d", p=128)
    y_pair = y_d.ap().rearrange("(h t p) d -> h p t d", t=2, p=128)

    with tile.TileContext(nc) as tc:
        with (
            tc.tile_pool(name="const", bufs=1) as cpool,
            tc.tile_pool(name="xp", bufs=4) as xpool,
            tc.tile_pool(name="xtp", bufs=2) as xtpool,
            tc.tile_pool(name="yp", bufs=4) as ypool,
            tc.tile_pool(name="small", bufs=6) as spool,
            tc.tile_pool(name="pst", bufs=4, space="PSUM") as pst,
            tc.tile_pool(name="psu", bufs=2, space="PSUM") as psu,
            tc.tile_pool(name="psr", bufs=2, space="PSUM") as psr,
        ):
            GTMAX = max(GROUPS)

            def load_group(base_t, gt):
                """DMA gt tiles starting at tile base_t into a group tile.

                Tile dtype is f32r (byte-identical to the f32 DMA payload) so
                the PE transposes can consume it directly; the DVE fuse reads
                it bitcast back to f32.
                """
                xg = xpool.tile([128, GTMAX, D], f32r, tag="xg")
                t = 0
                while t < gt:
                    if t + 2 <= gt and (base_t + t) % 2 == 0:
                        nc.gpsimd.dma_start(
                            out=xg[:, t : t + 2, :], in_=x_pair[(base_t + t) // 2]
                        )
                        t += 2
                    else:
                        nc.gpsimd.dma_start(out=xg[:, t, :], in_=x_tile[base_t + t])
                        t += 1
                return xg

            # --- first x data on the wire before anything else ---
            xg0 = load_group(0, GROUPS[0])

            # --- warmup: dense bf16 matmuls during initial DMA window ---
            dummy = cpool.tile([128, 512], bf16)
            nc.gpsimd.memset(dummy[:], 0.0)
            for i in range(N_WARM):
                pw = psr.tile([128, 512], f32, tag="psr")
                nc.tensor.matmul(pw[:], dummy[:, 0:128], dummy[:], start=True, stop=True)

            # --- constants ---
            ident = cpool.tile([128, 128], f32r)
            nc.gpsimd.dma_start(out=ident[:], in_=id_d[:, :])
            ident_r = ident[:]
            wt_sb = cpool.tile([128, NCH, L], f32r)
            nc.gpsimd.dma_start(out=wt_sb[:], in_=wt_d.ap().rearrange("p (c l) -> p c l", l=L))
            b7_sb = cpool.tile([128, D], f32)
            nc.sync.dma_start(out=b7_sb[:], in_=b7_d[:, :])
            # scan constants: cc[:, 0] = 1 (folds the +1 of rho_1), cc[:, l] = c_l
            cc_sb = cpool.tile([128, L], f32)
            nc.gpsimd.memset(cc_sb[:, 0:1], 1.0)
            for l in range(1, L):
                nc.gpsimd.memset(cc_sb[:, l : l + 1], cvals[l - 1])
            ones = cpool.tile([128, 1], f32)
            nc.gpsimd.memset(ones[:], 1.0)

            base_t = 0
            for g, gt in enumerate(GROUPS):
                if g == 0:
                    xg = xg0
                else:
                    xg = load_group(base_t, gt)
                xg_c = xg[:].rearrange("p t (c d) -> p t c d", c=NCH)

                # transpose the group's chunks -> xT [128d, c, gt*128 rows]
                xT = xtpool.tile([128, NCH, GTMAX * 128], f32r, tag="xT")
                for t in range(gt):
                    h = NCH // 2
                    pa = pst.tile([128, h, 128], f32, tag="pst")
                    for c in range(h):
                        nc.tensor.transpose(
                            pa[:, c, :].bitcast(f32r),
                            xg_c[:, t, c, :],
                            ident_r,
                        )
                    nc.scalar.copy(xT[:, 0:h, 128 * t : 128 * (t + 1)], pa[:].bitcast(f32r))
                    pb = pst.tile([128, h, 128], f32, tag="pst")
                    for c in range(h):
                        nc.tensor.transpose(
                            pb[:, c, :].bitcast(f32r),
                            xg_c[:, t, h + c, :],
                            ident_r,
                        )
                    nc.scalar.copy(xT[:, h:NCH, 128 * t : 128 * (t + 1)], pb[:].bitcast(f32r))

                # U^T for the whole group: [L, gt*128] = sum_c WT_c.T @ xT_c
                gw = gt * 128
                ps_u = psu.tile([L, GTMAX * 128], f32, tag="psu")
                for c in range(NCH):
                    nc.tensor.matmul(
                        ps_u[:, 0:gw],
                        wt_sb[:, c, :],
                        xT[:, c, 0:gw],
                        start=(c == 0), stop=(c == NCH - 1),
                    )
                ut = spool.tile([L, GTMAX * 128], f32r, tag="ut")
                nc.scalar.copy(ut[:, 0:gw], ps_u[:, 0:gw])

                for t in range(gt):
                    # U tile back to row-partition orientation: [128, L]
                    pr = psr.tile([128, L], f32, tag="psr")
                    nc.tensor.transpose(
                        pr[:].bitcast(f32r),
                        ut[:, 128 * t : 128 * (t + 1)],
                        ident_r[0:L, 0:L],
                    )
                    # rho chain: rho_{l+1} = rho_l * u_l + c_l, rho_0 = 1, c_0 = 1
                    scano = spool.tile([128, L], f32, tag="scan")
                    nc.vector.tensor_tensor_scan(
                        scano[:], pr[:], cc_sb[:], ones[:, 0:1],
                        mybir.AluOpType.mult, mybir.AluOpType.add,
                    )
                    # out = x * rho + b7
                    tt = base_t + t
                    if tt % 2 == 0:
                        yt = ypool.tile([128, 2, D], f32, tag="yt")
                    nc.vector.scalar_tensor_tensor(
                        yt[:, tt % 2, :], xg[:, t, :].bitcast(f32), scano[:, L - 1 : L],
                        b7_sb[:], mybir.AluOpType.mult, mybir.AluOpType.add,
                    )
                    if tt % 2 == 1:
                        nc.sync.dma_start(out=y_pair[tt // 2], in_=yt[:])
                base_t += gt

    nc.compile()
    return nc


def kernel(x, W, b):
    global LAST_RESULTS
    x = np.ascontiguousarray(np.asarray(x), dtype=np.float32)
    W = np.ascontiguousarray(np.asarray(W), dtype=np.float32)
    b = np.ascontiguousarray(np.asarray(b), dtype=np.float32)
    assert x.shape == (B, D) and W.shape == (L, D) and b.shape == (L, D)

    cvals = [float(np.dot(b[l - 1].astype(np.float64), W[l].astype(np.float64)) + 1.0)
             for l in range(1, L)]
    wt = W.T.reshape(NCH, 128, L).transpose(1, 0, 2).reshape(128, NCH * L)
    wt = np.ascontiguousarray(wt, dtype=np.float32)
    b7r = np.ascontiguousarray(np.broadcast_to(b[L - 1], (128, D)), dtype=np.float32)
    ident = np.eye(128, dtype=np.float32)

    nc = _build(cvals)

    shards = [x[i * RPC : (i + 1) * RPC] for i in range(N_CORES)]
    in_maps = [{"x": s, "wt": wt, "b7r": b7r, "ident": ident} for s in shards]
    res = run_bass_kernel_spmd(nc, in_maps, core_ids=list(range(N_CORES)))
    LAST_RESULTS = res
    out = np.concatenate([res.results[i]["y"] for i in range(N_CORES)], axis=0)
    return out.astype(np.float32)


# revision 26
# speedup vs baseline: 1.0377x; 1.0377x over previous
"""Trainium2 Bass kernel for nn_CrossLayer (DCN-style cross stack).

Reference semantics (B=16384, D=1024, L=8):
    out_0 = x
    s_i = einsum('bd,d->b', out_i, W[i])
    out_{i+1} = x * s_i[:, None] + b[i] + x

Algebraic collapse: out_{i+1} = x * rho_{i+1} + b[i] with
    rho_1 = u_0 + 1,   rho_{l+1} = rho_l * u_l + c_l
    u_l[r] = <x[r, :], W[l]>          (U = x @ W.T, [B, L])
    c_l = <b[l-1], W[l]> + 1          (weights-only scalars)
    out = x * rho_8[:, None] + b[L-1]

Device work: U = x @ W.T via PE transposes + W-stationary matmuls, all in
float32r (1-pass PE datapath: transpose 1.5 cyc/row, matmul 1 cyc/row at
>=256 moving cols, vs 2/4 for plain fp32), an 8-step per-row scan on DVE
(initial=1, c_0=1 folds the +1 into the scan), one fused scale+bias pass.

Memory layout: 256-row blocks where partition p holds DRAM rows 2p/2p+1
of the block -> every DMA descriptor is 8KB contiguous (vs 4KB for the
naive row-per-partition layout). The row permutation is never undone: the
transposes, scan, fuse, and the output DMA all use the same (p, slot)
mapping. x read once, out written once -> memory-roofline bound.

Sharding: data-parallel over batch; 8 cores x 2048 rows. Tiny (L, D)
weights replicated.
"""

import numpy as np

import concourse.bacc as bacc
import concourse.tile as tile
from concourse import mybir
from concourse.bass_utils import run_bass_kernel_spmd

N_CORES = 8
B, D, L = 16384, 1024, 8
RPC = B // N_CORES          # rows per core (2048)
NB = RPC // 256             # 256-row blocks per core (8)
NCH = D // 128              # 128-wide d chunks (8)
GROUPS = [2, 2, 2, 1, 1]    # blocks per group (sum = NB); small tail groups
N_WARM = 8                  # bf16 warmup matmuls to hold PE p-state up

LAST_RESULTS = None


def _build(cvals):
    """Trace + compile the per-core program. cvals = [c_1..c_{L-1}]."""
    nc = bacc.Bacc("TRN2", target_bir_lowering=False, debug=False)
    f32 = mybir.dt.float32
    f32r = mybir.dt.float32r
    bf16 = mybir.dt.bfloat16

    # x/wt/ident declared f32r (byte-identical to the f32 numpy payload) so
    # the sync engine can DMA them straight into f32r tiles (no cast) and
    # the BIR fp32r-producer check is satisfied.
    x_d = nc.dram_tensor("x", [RPC, D], f32r, kind="ExternalInput")
    wt_d = nc.dram_tensor("wt", [128, NCH * L], f32r, kind="ExternalInput")
    b7_d = nc.dram_tensor("b7r", [128, D], f32, kind="ExternalInput")
    id_d = nc.dram_tensor("ident", [128, 128], f32r, kind="ExternalInput")
    y_d = nc.dram_tensor("y", [RPC, D], f32, kind="ExternalOutput")

    # block views: partition p <-> rows 2p, 2p+1 of the block (8KB descr.)
    x_blk = x_d.ap().rearrange("(t p r) d -> t p (r d)", p=128, r=2)
    y_blk = y_d.ap().rearrange("(t p r) d -> t p (r d)", p=128, r=2)

    with tile.TileContext(nc) as tc:
        with (
            tc.tile_pool(name="const", bufs=1) as cpool,
            tc.tile_pool(name="xp", bufs=4) as xpool,
            tc.tile_pool(name="xtp", bufs=2) as xtpool,
            tc.tile_pool(name="yp", bufs=4) as ypool,
            tc.tile_pool(name="small", bufs=6) as spool,
            tc.tile_pool(name="pst", bufs=4, space="PSUM") as pst,
            tc.tile_pool(name="psu", bufs=2, space="PSUM") as psu,
            tc.tile_pool(name="psr", bufs=2, space="PSUM") as psr,
        ):
            GBMAX = max(GROUPS)

            def load_group(base_b, gb):
                """DMA gb 256-row blocks starting at block base_b."""
                xg = xpool.tile([128, GBMAX, 2 * D], f32r, tag="xg")
                for i in range(gb):
                    nc.sync.dma_start(out=xg[:, i, :], in_=x_blk[base_b + i])
                return xg

            # --- first x data on the wire before anything else ---
            xg0 = load_group(0, GROUPS[0])

            # --- warmup: dense bf16 matmuls during initial DMA window ---
            dummy = cpool.tile([128, 512], bf16)
            nc.gpsimd.memset(dummy[:], 0.0)
            for i in range(N_WARM):
                pw = psr.tile([128, 512], f32, tag="psr")
                nc.tensor.matmul(pw[:], dummy[:, 0:128], dummy[:], start=True, stop=True)

            # --- constants ---
            ident = cpool.tile([128, 128], f32r)
            nc.sync.dma_start(out=ident[:], in_=id_d[:, :])
            wt_sb = cpool.tile([128, NCH, L], f32r)
            nc.sync.dma_start(out=wt_sb[:], in_=wt_d.ap().rearrange("p (c l) -> p c l", l=L))
            b7_sb = cpool.tile([128, D], f32)
            nc.sync.dma_start(out=b7_sb[:], in_=b7_d[:, :])
            # scan constants: cc[:, 0] = 1 (folds the +1 of rho_1), cc[:, l] = c_l
            cc_sb = cpool.tile([128, L], f32)
            nc.gpsimd.memset(cc_sb[:, 0:1], 1.0)
            for l in range(1, L):
                nc.gpsimd.memset(cc_sb[:, l : l + 1], cvals[l - 1])
            ones = cpool.tile([128, 1], f32)
            nc.gpsimd.memset(ones[:], 1.0)

            base_b = 0
            for g, gb in enumerate(GROUPS):
                if g == 0:
                    xg = xg0
                else:
                    xg = load_group(base_b, gb)
                # [p, block, slot, chunk, 128]
                xg_c = xg[:].rearrange("p g (r c d) -> p g r c d", r=2, c=NCH)
                # [p, block, slot, 1024] for the fuse
                xg_f = xg[:].rearrange("p g (r d) -> p g r d", r=2)

                # transpose chunks -> xT [128d, c, gb*256 cols]; col = b*256+s*128+p
                xT = xtpool.tile([128, NCH, GBMAX * 256], f32r, tag="xT")
                for bs in range(2 * gb):
                    i, s = bs // 2, bs % 2
                    off = 128 * bs
                    h = NCH // 2
                    pa = pst.tile([128, h, 128], f32, tag="pst")
                    for c in range(h):
                        nc.tensor.transpose(
                            pa[:, c, :].bitcast(f32r), xg_c[:, i, s, c, :], ident[:]
                        )
                    nc.scalar.copy(xT[:, 0:h, off : off + 128], pa[:].bitcast(f32r))
                    pb = pst.tile([128, h, 128], f32, tag="pst")
                    for c in range(h):
                        nc.tensor.transpose(
                            pb[:, c, :].bitcast(f32r), xg_c[:, i, s, h + c, :], ident[:]
                        )
                    nc.scalar.copy(xT[:, h:NCH, off : off + 128], pb[:].bitcast(f32r))

                # U^T for the whole group: [L, gb*256] = sum_c WT_c.T @ xT_c
                gw = gb * 256
                ps_u = psu.tile([L, GBMAX * 256], f32, tag="psu")
                for c in range(NCH):
                    nc.tensor.matmul(
                        ps_u[:, 0:gw],
                        wt_sb[:, c, :],
                        xT[:, c, 0:gw],
                        start=(c == 0), stop=(c == NCH - 1),
                    )
                ut = spool.tile([L, GBMAX * 256], f32r, tag="ut")
                nc.scalar.copy(ut[:, 0:gw], ps_u[:, 0:gw])

                for i in range(gb):
                    yt = ypool.tile([128, 2, D], f32, tag="yt")
                    for s in range(2):
                        off = 128 * (2 * i + s)
                        # U slot back to row-partition orientation: [128, L]
                        pr = psr.tile([128, L], f32, tag="psr")
                        nc.tensor.transpose(
                            pr[:].bitcast(f32r),
                            ut[:, off : off + 128],
                            ident[0:L, 0:L],
                        )
                        # rho chain: rho_{l+1} = rho_l*u_l + c_l, rho_0 = c_0 = 1
                        scano = spool.tile([128, L], f32, tag="scan")
                        nc.vector.tensor_tensor_scan(
                            scano[:], pr[:], cc_sb[:], ones[:, 0:1],
                            mybir.AluOpType.mult, mybir.AluOpType.add,
                        )
                        # out = x * rho + b7
                        nc.vector.scalar_tensor_tensor(
                            yt[:, s, :], xg_f[:, i, s, :].bitcast(f32),
                            scano[:, L - 1 : L], b7_sb[:],
                            mybir.AluOpType.mult, mybir.AluOpType.add,
                        )
                    nc.gpsimd.dma_start(out=y_blk[base_b + i], in_=yt[:])
                base_b += gb

    nc.compile()
    return nc


def kernel(x, W, b):
    global LAST_RESULTS
    x = np.ascontiguousarray(np.asarray(x), dtype=np.float32)
    W = np.ascontiguousarray(np.asarray(W), dtype=np.float32)
    b = np.ascontiguousarray(np.asarray(b), dtype=np.float32)
    assert x.shape == (B, D) and W.shape == (L, D) and b.shape == (L, D)

    cvals = [float(np.dot(b[l - 1].astype(np.float64), W[l].astype(np.float64)) + 1.0)
             for l in range(1, L)]
    wt = W.T.reshape(NCH, 128, L).transpose(1, 0, 2).reshape(128, NCH * L)
    wt = np.ascontiguousarray(wt, dtype=np.float32)
    b7r = np.ascontiguousarray(np.broadcast_to(b[L - 1], (128, D)), dtype=np.float32)
    ident = np.eye(128, dtype=np.float32)

    nc = _build(cvals)

    shards = [x[i * RPC : (i + 1) * RPC] for i in range(N_CORES)]
    in_maps = [{"x": s, "wt": wt, "b7r": b7r, "ident": ident} for s in shards]
    res = run_bass_kernel_spmd(nc, in_maps, core_ids=list(range(N_CORES)))
    LAST_RESULTS = res
    out = np.concatenate([res.results[i]["y"] for i in range(N_CORES)], axis=0)
    return out.astype(np.float32)
